# revision 39
# baseline (speedup 1.0000x reference)
"""Distributed 3-layer GAT (PyG GATConv-style) for one TRN2 chip (8 NeuronCores).

Strategy (dst-node sharding):
  - Nodes are range-sharded across 8 cores (1250 each). Each core owns the
    softmax + aggregation for edges whose dst lands in its range.
  - Per layer, each core computes hw = h @ [W | Ws | Wd] for its node shard
    (Ws/Wd fold the attention dot-products into the matmul), packs rows as
    [512 x bf16 hw | s (4xf32) | d (4xf32) | pad] = 1280B, and an AllGather
    replicates the full 10000-row table into every core's HBM.
  - Each core then dma_gathers the rows for its incoming edges (by src id,
    <=1024 idxs per call: SWDGE ring cap; idx wrap replicated across the 8
    Q7 partition groups), expands d[dst] per edge via a TensorE matmul
    against a transposed compare-built indicator, computes
    ex = exp(leakyrelu(s_src + d_dst)) per edge (no max-subtraction needed:
    scores are bounded, softmax is shift-invariant per dst anyway),
    scales gathered features by ex, and reduces per dst-node with a
    TensorE matmul against a compare-built 0/1 indicator (edge -> dst-slot).
  - Divide by the segment sum of ex, bias/relu/residual, transpose back to
    feature-major for the next layer's matmul.
  - Mean-pool per graph = one-hot matmul + AllReduce, tiny classifier MLP.
"""

import sys

import numpy as np

if "/opt/trn_rl_repo" not in sys.path:
    sys.path.insert(0, "/opt/trn_rl_repo")

NC_CORES = 8
N_HEADS = 4
NEG_SLOPE = 0.2
NUM_GRAPHS = 64


# --------------------------------------------------------------------------
# Host-side preprocessing
# --------------------------------------------------------------------------

def _prep(x, edge_index, batch, params, n_cores=NC_CORES):
    """Build the config + per-core input maps from the full-size inputs."""
    N, F_IN = x.shape
    C = params["W0"].shape[1]          # 512
    H = N_HEADS
    HID = C // H
    G = NUM_GRAPHS
    assert N % n_cores == 0
    NPC = N // n_cores
    NBLK = -(-NPC // 128)
    ROW = C + 128                      # bf16 cols per table row (1280 B)
    assert (C * 2) % 256 == 0

    # self-loops are handled in the epilogue from local rows (saves ~6% of
    # gather descriptor generation, the critical-path Q7 cost)
    src = np.asarray(edge_index[0]).astype(np.int64)
    dst = np.asarray(edge_index[1]).astype(np.int64)

    # bin edges by (core, block)
    core_of = dst // NPC
    blk_of = (dst % NPC) // 128

    counts = np.zeros((n_cores, NBLK), np.int64)
    np.add.at(counts, (core_of, blk_of), 1)
    CBS = [max(1, int(-(-counts[:, b].max() // 128))) for b in range(NBLK)]
    CMAX = max(CBS)

    # per-core index arrays
    srcidx = np.zeros((n_cores, NBLK, 128, CMAX * 8), np.int16)
    dstloc = np.full((n_cores, NBLK, 128, CMAX), -1.0, np.float32)
    dstlocT = np.full((n_cores, NBLK, 1, CMAX * 128), -1.0, np.float32)

    order = np.lexsort((core_of * NBLK + blk_of,))  # stable by (core, block)
    so, do_, co, bo = src[order], dst[order], core_of[order], blk_of[order]
    pos = 0
    for c in range(n_cores):
        for b in range(NBLK):
            n_e = int(counts[c, b])
            es, ed = so[pos : pos + n_e], do_[pos : pos + n_e]
            pos += n_e
            nE = CBS[b] * 128
            s_pad = np.zeros(nE, np.int64)
            s_pad[:n_e] = es
            loc_pad = np.full(nE, -1.0, np.float32)  # dst-in-block slot
            loc_pad[:n_e] = (ed - c * NPC - b * 128).astype(np.float32)
            i = np.arange(nE)
            srcidx[c, b, i % 16, i // 16] = s_pad.astype(np.int16)
            dstloc[c, b, i % 128, i // 128] = loc_pad
            dstlocT[c, b, 0, :nE] = loc_pad
    assert pos == src.shape[0]
    # the 8 GpSimd Q7 cores each read their own 16-partition copy of the
    # index list -> replicate the [16, n] wrap into all 8 partition groups
    for k in range(1, 8):
        srcidx[:, :, 16 * k : 16 * (k + 1), :] = srcidx[:, :, 0:16, :]

    # extended weights: fold attention vectors into the matmul
    wexts = []
    for li in range(3):
        W = np.asarray(params[f"W{li}"], np.float32)
        a_s = np.asarray(params[f"a_src{li}"], np.float32)
        a_d = np.asarray(params[f"a_dst{li}"], np.float32)
        Wr = W.reshape(W.shape[0], H, HID)
        Ws = np.einsum("fhc,hc->fh", Wr, a_s)
        Wd = np.einsum("fhc,hc->fh", Wr, a_d)
        wexts.append(np.concatenate([W, Ws, Wd], axis=1).astype(np.float32))

    biases = np.stack(
        [np.asarray(params[f"b{i}"], np.float32) for i in range(3)]
    )  # [3, C]

    # pooling
    cnt = np.bincount(np.asarray(batch).astype(np.int64), minlength=G).astype(
        np.float32
    )
    invcnt = (1.0 / np.maximum(cnt, 1.0)).reshape(G, 1).astype(np.float32)
    NPAD = NBLK * 128
    pool_oh_full = np.zeros((N, G), np.float32)
    pool_oh_full[np.arange(N), np.asarray(batch).astype(np.int64)] = 1.0

    xT = np.ascontiguousarray(np.asarray(x, np.float32).T)  # [F_IN, N]

    in_maps = []
    for c in range(n_cores):
        xts = np.zeros((F_IN, NPAD), np.float32)
        xts[:, :NPC] = xT[:, c * NPC : (c + 1) * NPC]
        poh = np.zeros((NPAD, G), np.float32)
        poh[:NPC] = pool_oh_full[c * NPC : (c + 1) * NPC]
        in_maps.append(
            {
                "xT": xts,
                "W0ext": wexts[0],
                "W1ext": wexts[1],
                "W2ext": wexts[2],
                "biases": biases,
                "cW1": np.asarray(params["cW1"], np.float32),
                "cb1": np.asarray(params["cb1"], np.float32).reshape(1, -1),
                "cW2": np.asarray(params["cW2"], np.float32),
                "cb2": np.asarray(params["cb2"], np.float32).reshape(1, 1),
                "poolOH": poh,
                "invcnt": invcnt,
                "ident": np.eye(128, dtype=np.float32),
                "srcidx": srcidx[c],
                "dstloc": dstloc[c],
                "dstlocT": dstlocT[c],
            }
        )

    cfg = dict(
        N=N, F_IN=F_IN, C=C, H=H, HID=HID, G=G, n_cores=n_cores, NPC=NPC,
        NBLK=NBLK, CBS=CBS, CMAX=CMAX, ROW=ROW,
    )
    return cfg, in_maps


# --------------------------------------------------------------------------
# Bass program
# --------------------------------------------------------------------------

def _build(cfg, debug=False):
    import concourse.bacc as bacc
    import concourse.bass as bass
    import concourse.mybir as mybir
    import concourse.tile as tile

    f32 = mybir.dt.float32
    bf16 = mybir.dt.bfloat16
    i16 = mybir.dt.int16
    i32 = mybir.dt.int32
    AF = mybir.ActivationFunctionType
    OP = mybir.AluOpType

    N = cfg["N"]; F_IN = cfg["F_IN"]; C = cfg["C"]; H = cfg["H"]
    G = cfg["G"]; NCC = cfg["n_cores"]; NPC = cfg["NPC"]; NBLK = cfg["NBLK"]
    CBS = cfg["CBS"]; CMAX = cfg["CMAX"]; ROW = cfg["ROW"]
    EXT = C + 2 * H                    # 520
    KI = F_IN // 128                   # k-chunks layer 0
    KC = C // 128                      # k-chunks layers 1,2 / feature quarters
    RG = [list(range(NCC))]
    LASTV = NPC - (NBLK - 1) * 128     # valid rows in last block

    nc = bacc.Bacc("TRN2", target_bir_lowering=False, debug=False,
                   num_devices=NCC)

    # ---- external I/O ----
    xT_d = nc.dram_tensor("xT", [F_IN, NBLK * 128], f32, kind="ExternalInput")
    wext_d = [
        nc.dram_tensor(f"W{li}ext", [F_IN if li == 0 else C, EXT], f32,
                       kind="ExternalInput")
        for li in range(3)
    ]
    bias_d = nc.dram_tensor("biases", [3, C], f32, kind="ExternalInput")
    cW1_d = nc.dram_tensor("cW1", [C, 128], f32, kind="ExternalInput")
    cb1_d = nc.dram_tensor("cb1", [1, 128], f32, kind="ExternalInput")
    cW2_d = nc.dram_tensor("cW2", [128, 1], f32, kind="ExternalInput")
    cb2_d = nc.dram_tensor("cb2", [1, 1], f32, kind="ExternalInput")
    poolOH_d = nc.dram_tensor("poolOH", [NBLK * 128, G], f32, kind="ExternalInput")
    invcnt_d = nc.dram_tensor("invcnt", [G, 1], f32, kind="ExternalInput")
    ident_d = nc.dram_tensor("ident", [128, 128], f32, kind="ExternalInput")
    srcidx_d = nc.dram_tensor("srcidx", [NBLK, 128, CMAX * 8], i16,
                              kind="ExternalInput")
    dstloc_d = nc.dram_tensor("dstloc", [NBLK, 128, CMAX], f32,
                              kind="ExternalInput")
    dstlocT_d = nc.dram_tensor("dstlocT", [NBLK, 1, CMAX * 128], f32,
                               kind="ExternalInput")
    out_d = nc.dram_tensor("out", [G, C + 1], f32, kind="ExternalOutput")
    if debug:
        dbg_tbl = nc.dram_tensor("dbg_tbl", [N, ROW], bf16, kind="ExternalOutput")
        dbg_G = nc.dram_tensor("dbg_G", [128, CMAX * ROW], bf16,
                               kind="ExternalOutput")
        dbg_ex = nc.dram_tensor("dbg_ex", [128, CMAX * H], f32,
                                kind="ExternalOutput")
        dbg_ind = nc.dram_tensor("dbg_ind", [128, CMAX * 128], bf16,
                                 kind="ExternalOutput")
        dbg_h = nc.dram_tensor("dbg_h", [3, NBLK * 128, C], f32,
                               kind="ExternalOutput")

    # ---- internal DRAM ----
    tbl_sh = [nc.dram_tensor(f"tbl_sh{li}", [NPC, ROW], bf16) for li in range(3)]
    tbl = [
        nc.dram_tensor(f"tbl{li}", [N, ROW], bf16, addr_space="Shared")
        for li in range(3)
    ]
    pool_in = nc.dram_tensor("pool_in", [G, C], f32)
    pool_out = nc.dram_tensor("pool_out", [G, C], f32, addr_space="Shared")

    from contextlib import ExitStack

    with tile.TileContext(nc) as tc, ExitStack() as stk:
        const = stk.enter_context(tc.tile_pool(name="const", bufs=1))
        hpool = stk.enter_context(tc.tile_pool(name="hpool", bufs=1))
        work = stk.enter_context(tc.tile_pool(name="work", bufs=2))
        meta = stk.enter_context(tc.tile_pool(name="meta", bufs=4))
        psum = stk.enter_context(tc.tile_pool(name="psum", bufs=1, space="PSUM"))
        psum2 = stk.enter_context(tc.tile_pool(name="psum2", bufs=2, space="PSUM"))

        # ---------------- constants / weights ----------------
        iota_i = const.tile([128, 128], i32)
        nc.gpsimd.iota(iota_i[:], pattern=[[1, 128]], base=0, channel_multiplier=0)
        iota_f = const.tile([128, 128], f32)
        nc.vector.tensor_copy(iota_f[:], iota_i[:])
        iotac_i = const.tile([128, 1], i32)
        nc.gpsimd.iota(iotac_i[:], pattern=[[0, 1]], base=0, channel_multiplier=1)
        iotac_f = const.tile([128, 1], f32)
        nc.vector.tensor_copy(iotac_f[:], iotac_i[:])

        xT_t = []
        for k in range(KI):
            t = const.tile([128, NBLK * 128], bf16, tag=f"xt{k}")
            nc.gpsimd.dma_start(t[:], xT_d[k * 128 : (k + 1) * 128, :])
            xT_t.append(t)
        w0_tiles = []
        for k in range(KI):
            t = const.tile([128, EXT], bf16, tag=f"w0_{k}")
            nc.gpsimd.dma_start(t[:], wext_d[0][k * 128 : (k + 1) * 128, :])
            w0_tiles.append(t)

        ident_f = const.tile([128, 128], f32)
        nc.sync.dma_start(ident_f[:], ident_d[:])
        ident_b = const.tile([128, 128], bf16)
        nc.gpsimd.dma_start(ident_b[:], ident_d[:])  # cast f32->bf16

        bias_t = []
        for li in range(3):
            bt = const.tile([128, C], f32, tag=f"bias{li}")
            nc.gpsimd.dma_start(
                bt[:], bias_d[li : li + 1, :].to_broadcast([128, C])
            )
            bias_t.append(bt)

        wt = [w0_tiles]  # wt[li][k] : [128, EXT] bf16
        for li in range(1, 3):
            tiles = []
            for k in range(KC):
                t = const.tile([128, EXT], bf16, tag=f"w{li}_{k}")
                nc.gpsimd.dma_start(t[:], wext_d[li][k * 128 : (k + 1) * 128, :])
                tiles.append(t)
            wt.append(tiles)

        cW1_t = []
        for k in range(KC):
            t = const.tile([128, 128], f32, tag=f"cw1_{k}")
            nc.sync.dma_start(t[:], cW1_d[k * 128 : (k + 1) * 128, :])
            cW1_t.append(t)
        cb1_t = const.tile([G, 128], f32)
        nc.gpsimd.dma_start(cb1_t[:], cb1_d[0:1, :].to_broadcast([G, 128]))
        cW2_t = const.tile([128, 1], f32)
        nc.sync.dma_start(cW2_t[:], cW2_d[:])
        cb2_t = const.tile([G, 1], f32)
        nc.gpsimd.dma_start(cb2_t[:], cb2_d[0:1, :].to_broadcast([G, 1]))
        invcnt_t = const.tile([G, 1], f32)
        nc.sync.dma_start(invcnt_t[:], invcnt_d[:])

        poolOH_t = []
        for b in range(NBLK):
            t = const.tile([128, G], f32, tag=f"poh{b}")
            nc.sync.dma_start(t[:], poolOH_d[b * 128 : (b + 1) * 128, :])
            poolOH_t.append(t)

        # ---------------- helpers ----------------
        def hw_matmul_to_table(li, b, lhsT_tiles):
            """lhsT_tiles: list of [128,128] bf16 K-chunk tiles for block b.
            Computes hw rows and writes table shard + d-table for layer li."""
            pm = psum.tile([128, EXT], f32, tag="mm")
            kk = len(lhsT_tiles)
            for k in range(kk):
                nc.tensor.matmul(pm[:, :C], lhsT_tiles[k][:], wt[li][k][:, :C],
                                 start=(k == 0), stop=(k == kk - 1))
                nc.tensor.matmul(pm[:, C:EXT], lhsT_tiles[k][:],
                                 wt[li][k][:, C:EXT],
                                 start=(k == 0), stop=(k == kk - 1))
            rowt = work.tile([128, ROW], bf16, tag="rowt")
            nc.scalar.copy(rowt[:, :C], pm[:, :C])
            nc.vector.memset(rowt[:, C + 4 * H : ROW], 0.0)
            rf32 = rowt[:].bitcast(f32)
            nc.vector.tensor_copy(rf32[:, C // 2 : C // 2 + 2 * H], pm[:, C:EXT])
            nv = 128 if b < NBLK - 1 else LASTV
            nc.sync.dma_start(tbl_sh[li][b * 128 : b * 128 + nv, :], rowt[:nv, :])
            dsb = hpool.tile([128, H], bf16, tag=f"d{li}_{b}")
            nc.vector.tensor_copy(dsb[:], pm[:, C + H : EXT])
            d_sb[(li, b)] = dsb
            ssb = hpool.tile([128, H], bf16, tag=f"s{li}_{b}")
            nc.vector.tensor_copy(ssb[:], pm[:, C : C + H])
            s_sb[(li, b)] = ssb

        def maybe_allgather(li, b):
            if b == NBLK - 1:
                nc.gpsimd.collective_compute(
                    "AllGather", OP.bypass, replica_groups=RG,
                    ins=[tbl_sh[li][:, :]], outs=[tbl[li][:, :]],
                )

        d_sb = {}
        s_sb = {}

        # ---------------- layer 0 projection ----------------
        for b in range(NBLK):
            lhs = [xT_t[k][:, b * 128 : (b + 1) * 128] for k in range(KI)]
            hw_matmul_to_table(0, b, lhs)
            maybe_allgather(0, b)
        if debug:
            nc.sync.dma_start(dbg_tbl[:, :], tbl[0][:, :])

        # ---------------- GAT layers ----------------
        h_tiles = {}  # (layer, block) -> row-major [128, C] f32 tile
        for li in range(3):
            for b in range(NBLK):
                CB = CBS[b]
                nE = CB * 128
                sidx = meta.tile([128, CMAX * 8], i16, tag="sidx")
                nc.sync.dma_start(sidx[:, : CB * 8], srcidx_d[b, :, : CB * 8])
                dloc = meta.tile([128, CMAX], f32, tag="dloc")
                nc.sync.dma_start(dloc[:, :CB], dstloc_d[b, :, :CB])
                dlocT = work.tile([128, CMAX * 128], f32, tag="dlocT")
                nc.sync.dma_start(
                    dlocT[:, : CB * 128],
                    dstlocT_d[b, 0:1, : CB * 128].to_broadcast([128, CB * 128]),
                )

                # transposed indicator [dst slot -> edge] for the d-expand
                ind_dm = work.tile([128, CMAX, 128], bf16, tag="inddm")
                nc.vector.tensor_scalar(
                    ind_dm[:, :CB, :],
                    dlocT[:, : CB * 128].rearrange("p (c j) -> p c j", j=128),
                    iotac_f[:, 0:1], None, op0=OP.is_equal,
                )
                pdx = psum.tile([128, CMAX * H], f32, tag="dexp")
                for c in range(CB):
                    nc.tensor.matmul(pdx[:, c * H : (c + 1) * H],
                                     ind_dm[:, c, :], d_sb[(li, b)][:],
                                     start=True, stop=True)

                # indicator [edge -> dst slot]
                ind = work.tile([128, CMAX, 128], bf16, tag="ind")
                nc.vector.tensor_tensor(
                    ind[:, :CB, :],
                    iota_f[:, None, :].to_broadcast([128, CB, 128]),
                    dloc[:, :CB, None].to_broadcast([128, CB, 128]),
                    op=OP.is_equal,
                )

                # gather + per-sub-range scores/scale (SWDGE ring caps one
                # gather at 1024 idxs; each 8-chunk slice proceeds to
                # scores -> scale as soon as its gather lands)
                # sub-gather split: stay under the 1024-idx ring max
                # (896) and issue the small remainder first so the first
                # chunk's compute chain starts earliest
                GMAX = 6
                sizes = [((CB - 1) % GMAX) + 1]
                while sum(sizes) < CB:
                    sizes.append(GMAX)
                Gt = work.tile([128, CMAX, ROW], bf16, tag="G")
                Gf32 = Gt[:].bitcast(f32)
                exf = work.tile([128, CMAX, H], f32, tag="exf")
                exb = work.tile([128, CMAX, H], bf16, tag="exb")
                tmp = work.tile([128, CMAX, H], f32, tag="tmpe")
                c0 = 0
                for cw in sizes:
                    sl = slice(c0, c0 + cw)
                    nc.gpsimd.dma_gather(
                        Gt[:, sl, :], tbl[li][:, :],
                        sidx[:, c0 * 8 : (c0 + cw) * 8],
                        num_idxs=cw * 128, num_idxs_reg=cw * 128,
                        elem_size=ROW,
                    )
                    nc.vector.tensor_tensor(
                        exf[:, sl, :], Gf32[:, sl, C // 2 : C // 2 + H],
                        pdx[:, c0 * H : (c0 + cw) * H].rearrange(
                            "p (c h) -> p c h", h=H),
                        op=OP.add,
                    )
                    nc.scalar.mul(tmp[:, sl, :], exf[:, sl, :], NEG_SLOPE)
                    nc.vector.tensor_tensor(exf[:, sl, :], exf[:, sl, :],
                                            tmp[:, sl, :], op=OP.max)
                    nc.scalar.activation(exf[:, sl, :], exf[:, sl, :], AF.Exp)
                    nc.scalar.copy(exb[:, sl, :], exf[:, sl, :])
                    Gh = Gt[:, sl, :C].rearrange("p c (h f) -> p c h f",
                                                 f=C // H)
                    exv = exb[:, sl, :, None].to_broadcast(
                        [128, cw, H, C // H])
                    nc.vector.tensor_tensor(Gh, Gh, exv, op=OP.mult)
                    c0 += cw

                if debug and li == 0 and b == 0:
                    nc.sync.dma_start(dbg_G[:, :], Gt[:, :, :])
                    nc.sync.dma_start(dbg_ex[:, :], exf[:, :, :])
                    nc.sync.dma_start(dbg_ind[:, :], ind[:, :, :])

                # aggregate (den shares the agg tile's second PSUM bank)
                pagg = psum2.tile([128, C + H], f32, tag="agg")
                pa = pagg[:, :C]
                pd = pagg[:, C : C + H]
                for c in range(CB):
                    nc.tensor.matmul(pa, ind[:, c, :], Gt[:, c, :C],
                                     start=(c == 0), stop=(c == CB - 1))
                    nc.tensor.matmul(pd, ind[:, c, :], exb[:, c, :],
                                     start=(c == 0), stop=(c == CB - 1))

                # epilogue: fold in the self-loop term, then
                # h_new = relu(num/den + bias) (+ residual)
                nv = 128 if b < NBLK - 1 else LASTV
                exs = work.tile([128, H], f32, tag="exs")
                nc.vector.tensor_tensor(exs[:], s_sb[(li, b)][:],
                                        d_sb[(li, b)][:], op=OP.add)
                tmps = work.tile([128, H], f32, tag="tmps")
                nc.scalar.mul(tmps[:], exs[:], NEG_SLOPE)
                nc.vector.tensor_tensor(exs[:], exs[:], tmps[:], op=OP.max)
                nc.scalar.activation(exs[:], exs[:], AF.Exp)
                exs_b = work.tile([128, H], bf16, tag="exsb")
                nc.scalar.copy(exs_b[:], exs[:])
                hw_own = work.tile([128, C], bf16, tag="hwown")
                nc.vector.memset(hw_own[:], 0.0)
                nc.sync.dma_start(
                    hw_own[:nv, :],
                    tbl_sh[li][b * 128 : b * 128 + nv, :C],
                )
                selfm = work.tile([128, C], f32, tag="selfm")
                hwv = hw_own[:].rearrange("p (h f) -> p h f", f=C // H)
                exv2 = exs_b[:, :, None].to_broadcast([128, H, C // H])
                nc.vector.tensor_tensor(
                    selfm[:].rearrange("p (h f) -> p h f", f=C // H),
                    hwv, exv2, op=OP.mult,
                )
                nc.vector.tensor_tensor(selfm[:], selfm[:], pa, op=OP.add)
                rden = work.tile([128, H], f32, tag="rden")
                nc.vector.tensor_tensor(rden[:], pd, exs[:], op=OP.add)
                nc.vector.reciprocal(rden[:], rden[:])
                hn = hpool.tile([128, C], f32, tag=f"h{li}_{b}")
                HIDQ = C // H
                for h in range(H):
                    nc.scalar.mul(
                        hn[:, h * HIDQ : (h + 1) * HIDQ],
                        selfm[:, h * HIDQ : (h + 1) * HIDQ],
                        rden[:, h : h + 1],
                    )
                nc.vector.tensor_tensor(hn[:], hn[:], bias_t[li][:], op=OP.add)
                nc.scalar.activation(hn[:], hn[:], AF.Relu)
                if li > 0:
                    nc.vector.tensor_tensor(hn[:], hn[:],
                                            h_tiles[(li - 1, b)][:], op=OP.add)
                h_tiles[(li, b)] = hn
                if debug:
                    nc.sync.dma_start(dbg_h[li, b * 128 : (b + 1) * 128, :], hn[:])

                if li < 2:
                    # transpose to feature-major & next-layer projection
                    hb = work.tile([128, C], bf16, tag="hb16")
                    nc.scalar.copy(hb[:], hn[:])
                    hT = work.tile([128, KC, 128], bf16, tag="hT")
                    for f in range(KC):
                        pt = psum.tile([128, 128], bf16, tag="tr")
                        nc.tensor.transpose(
                            pt[:], hb[:, f * 128 : (f + 1) * 128], ident_b[:]
                        )
                        nc.scalar.copy(hT[:, f, :], pt[:])
                    hw_matmul_to_table(li + 1, b,
                                       [hT[:, f, :] for f in range(KC)])
                    maybe_allgather(li + 1, b)

        # ---------------- pooling + classifier ----------------
        # (rows beyond NPC in the last block hold finite garbage; their
        #  pool_oh one-hot rows are zero so they contribute nothing)
        pp = psum.tile([G, C], f32, tag="mm")
        for b in range(NBLK):
            nc.tensor.matmul(pp[:], poolOH_t[b][:], h_tiles[(2, b)][:],
                             start=(b == 0), stop=(b == NBLK - 1))
        sums = work.tile([G, C], f32, tag="sums")
        nc.vector.tensor_copy(sums[:], pp[:])
        nc.gpsimd.dma_start(pool_in[:, :], sums[:])
        nc.gpsimd.collective_compute(
            "AllReduce", OP.add, replica_groups=RG,
            ins=[pool_in[:, :]], outs=[pool_out[:, :]],
        )
        sfull = work.tile([G, C], f32, tag="sfull")
        nc.gpsimd.dma_start(sfull[:], pool_out[:, :])
        hp = work.tile([G, C], f32, tag="hp")
        nc.vector.tensor_scalar_mul(hp[:], sfull[:], invcnt_t[:, 0:1])

        hpT = work.tile([128, KC, G], f32, tag="hpT")
        for f in range(KC):
            pt = psum.tile([128, G], f32, tag="tr")
            nc.tensor.transpose(pt[:], hp[:, f * 128 : (f + 1) * 128],
                                ident_f[:G, :G])
            nc.vector.tensor_copy(hpT[:, f, :], pt[:])
        pc1 = psum.tile([G, 128], f32, tag="tr")
        for f in range(KC):
            nc.tensor.matmul(pc1[:], hpT[:, f, :], cW1_t[f][:],
                             start=(f == 0), stop=(f == KC - 1))
        z1 = work.tile([G, 128], f32, tag="z1")
        nc.vector.tensor_tensor(z1[:], pc1[:], cb1_t[:], op=OP.add)
        nc.scalar.activation(z1[:], z1[:], AF.Relu)
        pt2 = psum.tile([128, G], f32, tag="tr")
        nc.tensor.transpose(pt2[:], z1[:], ident_f[:G, :G])
        z1T = work.tile([128, G], f32, tag="z1T")
        nc.vector.tensor_copy(z1T[:], pt2[:])
        po = psum.tile([G, 1], f32, tag="dexp")
        nc.tensor.matmul(po[:], z1T[:], cW2_t[:], start=True, stop=True)
        outc = work.tile([G, 1], f32, tag="outc")
        nc.vector.tensor_scalar(outc[:], po[:], cb2_t[:, 0:1], None, op0=OP.add)

        nc.sync.dma_start(out_d[:, 0:1], outc[:])
        nc.sync.dma_start(out_d[:, 1 : C + 1], hp[:])

    return nc


# --------------------------------------------------------------------------
# Entry point
# --------------------------------------------------------------------------

def kernel(**inputs):
    x = np.asarray(inputs["x"], np.float32)
    edge_index = np.asarray(inputs["edge_index"])
    batch = np.asarray(inputs["batch"])
    cfg, in_maps = _prep(x, edge_index, batch, inputs)
    nc = _build(cfg)
    nc.compile()

    from concourse import bass_utils

    res = bass_utils.run_bass_kernel_spmd(
        nc, in_maps, core_ids=list(range(cfg["n_cores"]))
    ).results
    o = np.asarray(res[0]["out"], np.float32)
    out = np.ascontiguousarray(o[:, 0:1])
    hp = np.ascontiguousarray(o[:, 1 : cfg["C"] + 1])
    return out, hp


# revision 40
# speedup vs baseline: 1.1546x; 1.1546x over previous
"""Distributed 3-layer GAT (PyG GATConv-style) for one TRN2 chip (8 NeuronCores).

Strategy (dst-node sharding):
  - Nodes are range-sharded across 8 cores (1250 each). Each core owns the
    softmax + aggregation for edges whose dst lands in its range.
  - Per layer, each core computes hw = h @ [W | Ws | Wd] for its node shard
    (Ws/Wd fold the attention dot-products into the matmul), packs rows as
    [512 x bf16 hw | s (4xf32) | d (4xf32) | pad] = 1280B, and an AllGather
    replicates the full 10000-row table into every core's HBM.
  - Each core then dma_gathers the rows for its incoming edges (by src id,
    <=1024 idxs per call: SWDGE ring cap; idx wrap replicated across the 8
    Q7 partition groups), expands d[dst] per edge via a TensorE matmul
    against a transposed compare-built indicator, computes
    ex = exp(leakyrelu(s_src + d_dst)) per edge (no max-subtraction needed:
    scores are bounded, softmax is shift-invariant per dst anyway),
    scales gathered features by ex, and reduces per dst-node with a
    TensorE matmul against a compare-built 0/1 indicator (edge -> dst-slot).
  - Divide by the segment sum of ex, bias/relu/residual, transpose back to
    feature-major for the next layer's matmul.
  - Mean-pool per graph = one-hot matmul + AllReduce, tiny classifier MLP.
"""

import sys

import numpy as np

if "/opt/trn_rl_repo" not in sys.path:
    sys.path.insert(0, "/opt/trn_rl_repo")

NC_CORES = 8
N_HEADS = 4
NEG_SLOPE = 0.2
NUM_GRAPHS = 64


# --------------------------------------------------------------------------
# Host-side preprocessing
# --------------------------------------------------------------------------

def _prep(x, edge_index, batch, params, n_cores=NC_CORES):
    """Build the config + per-core input maps from the full-size inputs."""
    N, F_IN = x.shape
    C = params["W0"].shape[1]          # 512
    H = N_HEADS
    HID = C // H
    G = NUM_GRAPHS
    assert N % n_cores == 0
    NPC = N // n_cores
    NBLK = -(-NPC // 128)
    ROW = C + 128                      # bf16 cols per table row (1280 B)
    assert (C * 2) % 256 == 0

    # self-loops are handled in the epilogue from local rows (saves ~6% of
    # gather descriptor generation, the critical-path Q7 cost)
    src = np.asarray(edge_index[0]).astype(np.int64)
    dst = np.asarray(edge_index[1]).astype(np.int64)

    # bin edges by (core, block)
    core_of = dst // NPC
    blk_of = (dst % NPC) // 128

    counts = np.zeros((n_cores, NBLK), np.int64)
    np.add.at(counts, (core_of, blk_of), 1)
    CBS = [max(1, int(-(-counts[:, b].max() // 128))) for b in range(NBLK)]
    CMAX = max(CBS)

    # per-core index arrays
    srcidx = np.zeros((n_cores, NBLK, 128, CMAX * 8), np.int16)
    dstloc = np.full((n_cores, NBLK, 128, CMAX), -1.0, np.float32)
    dstlocT = np.full((n_cores, NBLK, 1, CMAX * 128), -1.0, np.float32)

    order = np.lexsort((core_of * NBLK + blk_of,))  # stable by (core, block)
    so, do_, co, bo = src[order], dst[order], core_of[order], blk_of[order]
    pos = 0
    for c in range(n_cores):
        for b in range(NBLK):
            n_e = int(counts[c, b])
            es, ed = so[pos : pos + n_e], do_[pos : pos + n_e]
            pos += n_e
            nE = CBS[b] * 128
            s_pad = np.zeros(nE, np.int64)
            s_pad[:n_e] = es
            loc_pad = np.full(nE, -1.0, np.float32)  # dst-in-block slot
            loc_pad[:n_e] = (ed - c * NPC - b * 128).astype(np.float32)
            i = np.arange(nE)
            srcidx[c, b, i % 16, i // 16] = s_pad.astype(np.int16)
            dstloc[c, b, i % 128, i // 128] = loc_pad
            dstlocT[c, b, 0, :nE] = loc_pad
    assert pos == src.shape[0]
    # the 8 GpSimd Q7 cores each read their own 16-partition copy of the
    # index list -> replicate the [16, n] wrap into all 8 partition groups
    for k in range(1, 8):
        srcidx[:, :, 16 * k : 16 * (k + 1), :] = srcidx[:, :, 0:16, :]

    # extended weights: fold attention vectors into the matmul
    wexts = []
    for li in range(3):
        W = np.asarray(params[f"W{li}"], np.float32)
        a_s = np.asarray(params[f"a_src{li}"], np.float32)
        a_d = np.asarray(params[f"a_dst{li}"], np.float32)
        Wr = W.reshape(W.shape[0], H, HID)
        Ws = np.einsum("fhc,hc->fh", Wr, a_s)
        Wd = np.einsum("fhc,hc->fh", Wr, a_d)
        wexts.append(np.concatenate([W, Ws, Wd], axis=1).astype(np.float32))

    biases = np.stack(
        [np.asarray(params[f"b{i}"], np.float32) for i in range(3)]
    )  # [3, C]

    # pooling
    cnt = np.bincount(np.asarray(batch).astype(np.int64), minlength=G).astype(
        np.float32
    )
    invcnt = (1.0 / np.maximum(cnt, 1.0)).reshape(G, 1).astype(np.float32)
    NPAD = NBLK * 128
    pool_oh_full = np.zeros((N, G), np.float32)
    pool_oh_full[np.arange(N), np.asarray(batch).astype(np.int64)] = 1.0

    xT = np.ascontiguousarray(np.asarray(x, np.float32).T)  # [F_IN, N]

    in_maps = []
    for c in range(n_cores):
        xts = np.zeros((F_IN, NPAD), np.float32)
        xts[:, :NPC] = xT[:, c * NPC : (c + 1) * NPC]
        poh = np.zeros((NPAD, G), np.float32)
        poh[:NPC] = pool_oh_full[c * NPC : (c + 1) * NPC]
        in_maps.append(
            {
                "xT": xts,
                "W0ext": wexts[0],
                "W1ext": wexts[1],
                "W2ext": wexts[2],
                "biases": biases,
                "cW1": np.asarray(params["cW1"], np.float32),
                "cb1": np.asarray(params["cb1"], np.float32).reshape(1, -1),
                "cW2": np.asarray(params["cW2"], np.float32),
                "cb2": np.asarray(params["cb2"], np.float32).reshape(1, 1),
                "poolOH": poh,
                "invcnt": invcnt,
                "ident": np.eye(128, dtype=np.float32),
                "srcidx": srcidx[c],
                "dstloc": dstloc[c],
                "dstlocT": dstlocT[c],
            }
        )

    cfg = dict(
        N=N, F_IN=F_IN, C=C, H=H, HID=HID, G=G, n_cores=n_cores, NPC=NPC,
        NBLK=NBLK, CBS=CBS, CMAX=CMAX, ROW=ROW,
    )
    return cfg, in_maps


# --------------------------------------------------------------------------
# Bass program
# --------------------------------------------------------------------------

def _build(cfg, debug=False):
    import concourse.bacc as bacc
    import concourse.bass as bass
    import concourse.mybir as mybir
    import concourse.tile as tile

    f32 = mybir.dt.float32
    bf16 = mybir.dt.bfloat16
    i16 = mybir.dt.int16
    i32 = mybir.dt.int32
    AF = mybir.ActivationFunctionType
    OP = mybir.AluOpType

    N = cfg["N"]; F_IN = cfg["F_IN"]; C = cfg["C"]; H = cfg["H"]
    G = cfg["G"]; NCC = cfg["n_cores"]; NPC = cfg["NPC"]; NBLK = cfg["NBLK"]
    CBS = cfg["CBS"]; CMAX = cfg["CMAX"]; ROW = cfg["ROW"]
    EXT = C + 2 * H                    # 520
    KI = F_IN // 128                   # k-chunks layer 0
    KC = C // 128                      # k-chunks layers 1,2 / feature quarters
    RG = [list(range(NCC))]
    LASTV = NPC - (NBLK - 1) * 128     # valid rows in last block

    nc = bacc.Bacc("TRN2", target_bir_lowering=False, debug=False,
                   num_devices=NCC)

    # ---- external I/O ----
    xT_d = nc.dram_tensor("xT", [F_IN, NBLK * 128], f32, kind="ExternalInput")
    wext_d = [
        nc.dram_tensor(f"W{li}ext", [F_IN if li == 0 else C, EXT], f32,
                       kind="ExternalInput")
        for li in range(3)
    ]
    bias_d = nc.dram_tensor("biases", [3, C], f32, kind="ExternalInput")
    cW1_d = nc.dram_tensor("cW1", [C, 128], f32, kind="ExternalInput")
    cb1_d = nc.dram_tensor("cb1", [1, 128], f32, kind="ExternalInput")
    cW2_d = nc.dram_tensor("cW2", [128, 1], f32, kind="ExternalInput")
    cb2_d = nc.dram_tensor("cb2", [1, 1], f32, kind="ExternalInput")
    poolOH_d = nc.dram_tensor("poolOH", [NBLK * 128, G], f32, kind="ExternalInput")
    invcnt_d = nc.dram_tensor("invcnt", [G, 1], f32, kind="ExternalInput")
    ident_d = nc.dram_tensor("ident", [128, 128], f32, kind="ExternalInput")
    srcidx_d = nc.dram_tensor("srcidx", [NBLK, 128, CMAX * 8], i16,
                              kind="ExternalInput")
    dstloc_d = nc.dram_tensor("dstloc", [NBLK, 128, CMAX], f32,
                              kind="ExternalInput")
    dstlocT_d = nc.dram_tensor("dstlocT", [NBLK, 1, CMAX * 128], f32,
                               kind="ExternalInput")
    out_d = nc.dram_tensor("out", [G, C + 1], f32, kind="ExternalOutput")
    if debug:
        dbg_tbl = nc.dram_tensor("dbg_tbl", [N, ROW], bf16, kind="ExternalOutput")
        dbg_G = nc.dram_tensor("dbg_G", [128, CMAX * ROW], bf16,
                               kind="ExternalOutput")
        dbg_ex = nc.dram_tensor("dbg_ex", [128, CMAX * H], f32,
                                kind="ExternalOutput")
        dbg_ind = nc.dram_tensor("dbg_ind", [128, CMAX * 128], bf16,
                                 kind="ExternalOutput")
        dbg_h = nc.dram_tensor("dbg_h", [3, NBLK * 128, C], f32,
                               kind="ExternalOutput")

    # ---- internal DRAM ----
    tbl_sh = [nc.dram_tensor(f"tbl_sh{li}", [NPC, ROW], bf16) for li in range(3)]
    tbl = [
        nc.dram_tensor(f"tbl{li}", [N, ROW], bf16, addr_space="Shared")
        for li in range(3)
    ]
    pool_in = nc.dram_tensor("pool_in", [G, C], f32)
    pool_out = nc.dram_tensor("pool_out", [G, C], f32, addr_space="Shared")

    from contextlib import ExitStack

    with tile.TileContext(nc) as tc, ExitStack() as stk:
        const = stk.enter_context(tc.tile_pool(name="const", bufs=1))
        hpool = stk.enter_context(tc.tile_pool(name="hpool", bufs=1))
        work = stk.enter_context(tc.tile_pool(name="work", bufs=2))
        meta = stk.enter_context(tc.tile_pool(name="meta", bufs=4))
        psum = stk.enter_context(tc.tile_pool(name="psum", bufs=1, space="PSUM"))
        psum2 = stk.enter_context(tc.tile_pool(name="psum2", bufs=2, space="PSUM"))

        # ---------------- constants / weights ----------------
        iota_i = const.tile([128, 128], i32)
        nc.gpsimd.iota(iota_i[:], pattern=[[1, 128]], base=0, channel_multiplier=0)
        iota_f = const.tile([128, 128], f32)
        nc.vector.tensor_copy(iota_f[:], iota_i[:])
        iotac_i = const.tile([128, 1], i32)
        nc.gpsimd.iota(iotac_i[:], pattern=[[0, 1]], base=0, channel_multiplier=1)
        iotac_f = const.tile([128, 1], f32)
        nc.vector.tensor_copy(iotac_f[:], iotac_i[:])

        ident_f = const.tile([128, 128], f32)
        nc.sync.dma_start(ident_f[:], ident_d[:])
        ident_b = const.tile([128, 128], bf16)
        nc.gpsimd.dma_start(ident_b[:], ident_d[:])  # cast f32->bf16

        bias_t = []
        for li in range(3):
            bt = const.tile([128, C], f32, tag=f"bias{li}")
            nc.gpsimd.dma_start(
                bt[:], bias_d[li : li + 1, :].to_broadcast([128, C])
            )
            bias_t.append(bt)

        wt = []  # wt[li][k] : [128, EXT] bf16
        for li in range(3):
            kk = KI if li == 0 else KC
            tiles = []
            for k in range(kk):
                t = const.tile([128, EXT], bf16, tag=f"w{li}_{k}")
                nc.gpsimd.dma_start(t[:], wext_d[li][k * 128 : (k + 1) * 128, :])
                tiles.append(t)
            wt.append(tiles)

        cW1_t = []
        for k in range(KC):
            t = const.tile([128, 128], f32, tag=f"cw1_{k}")
            nc.sync.dma_start(t[:], cW1_d[k * 128 : (k + 1) * 128, :])
            cW1_t.append(t)
        cb1_t = const.tile([G, 128], f32)
        nc.gpsimd.dma_start(cb1_t[:], cb1_d[0:1, :].to_broadcast([G, 128]))
        cW2_t = const.tile([128, 1], f32)
        nc.sync.dma_start(cW2_t[:], cW2_d[:])
        cb2_t = const.tile([G, 1], f32)
        nc.gpsimd.dma_start(cb2_t[:], cb2_d[0:1, :].to_broadcast([G, 1]))
        invcnt_t = const.tile([G, 1], f32)
        nc.sync.dma_start(invcnt_t[:], invcnt_d[:])

        poolOH_t = []
        for b in range(NBLK):
            t = const.tile([128, G], f32, tag=f"poh{b}")
            nc.sync.dma_start(t[:], poolOH_d[b * 128 : (b + 1) * 128, :])
            poolOH_t.append(t)

        xT_t = []
        for k in range(KI):
            t = const.tile([128, NBLK * 128], bf16, tag=f"xt{k}")
            nc.gpsimd.dma_start(t[:], xT_d[k * 128 : (k + 1) * 128, :])
            xT_t.append(t)

        # ---------------- helpers ----------------
        def hw_matmul_to_table(li, b, lhsT_tiles):
            """lhsT_tiles: list of [128,128] bf16 K-chunk tiles for block b.
            Computes hw rows and writes table shard + d-table for layer li."""
            pm = psum.tile([128, EXT], f32, tag="mm")
            kk = len(lhsT_tiles)
            for k in range(kk):
                nc.tensor.matmul(pm[:, :C], lhsT_tiles[k][:], wt[li][k][:, :C],
                                 start=(k == 0), stop=(k == kk - 1))
                nc.tensor.matmul(pm[:, C:EXT], lhsT_tiles[k][:],
                                 wt[li][k][:, C:EXT],
                                 start=(k == 0), stop=(k == kk - 1))
            rowt = work.tile([128, ROW], bf16, tag="rowt")
            nc.scalar.copy(rowt[:, :C], pm[:, :C])
            nc.vector.memset(rowt[:, C + 4 * H : ROW], 0.0)
            rf32 = rowt[:].bitcast(f32)
            nc.vector.tensor_copy(rf32[:, C // 2 : C // 2 + 2 * H], pm[:, C:EXT])
            nv = 128 if b < NBLK - 1 else LASTV
            nc.sync.dma_start(tbl_sh[li][b * 128 : b * 128 + nv, :], rowt[:nv, :])
            dsb = hpool.tile([128, H], bf16, tag=f"d{li}_{b}")
            nc.vector.tensor_copy(dsb[:], pm[:, C + H : EXT])
            d_sb[(li, b)] = dsb
            ssb = hpool.tile([128, H], bf16, tag=f"s{li}_{b}")
            nc.vector.tensor_copy(ssb[:], pm[:, C : C + H])
            s_sb[(li, b)] = ssb

        def maybe_allgather(li, b):
            if b == NBLK - 1:
                nc.gpsimd.collective_compute(
                    "AllGather", OP.bypass, replica_groups=RG,
                    ins=[tbl_sh[li][:, :]], outs=[tbl[li][:, :]],
                )

        d_sb = {}
        s_sb = {}

        # ---------------- layer 0 projection ----------------
        for b in range(NBLK):
            lhs = [xT_t[k][:, b * 128 : (b + 1) * 128] for k in range(KI)]
            hw_matmul_to_table(0, b, lhs)
            maybe_allgather(0, b)
        if debug:
            nc.sync.dma_start(dbg_tbl[:, :], tbl[0][:, :])

        # ---------------- GAT layers ----------------
        h_tiles = {}  # (layer, block) -> row-major [128, C] f32 tile
        for li in range(3):
            for b in range(NBLK):
                CB = CBS[b]
                nE = CB * 128
                sidx = meta.tile([128, CMAX * 8], i16, tag="sidx")
                nc.sync.dma_start(sidx[:, : CB * 8], srcidx_d[b, :, : CB * 8])
                dloc = meta.tile([128, CMAX], f32, tag="dloc")
                nc.sync.dma_start(dloc[:, :CB], dstloc_d[b, :, :CB])
                dlocT = work.tile([128, CMAX * 128], f32, tag="dlocT")
                nc.sync.dma_start(
                    dlocT[:, : CB * 128],
                    dstlocT_d[b, 0:1, : CB * 128].to_broadcast([128, CB * 128]),
                )

                # transposed indicator [dst slot -> edge] for the d-expand
                ind_dm = work.tile([128, CMAX, 128], bf16, tag="inddm")
                nc.vector.tensor_scalar(
                    ind_dm[:, :CB, :],
                    dlocT[:, : CB * 128].rearrange("p (c j) -> p c j", j=128),
                    iotac_f[:, 0:1], None, op0=OP.is_equal,
                )
                pdx = psum.tile([128, CMAX * H], f32, tag="dexp")
                for c in range(CB):
                    nc.tensor.matmul(pdx[:, c * H : (c + 1) * H],
                                     ind_dm[:, c, :], d_sb[(li, b)][:],
                                     start=True, stop=True)

                # indicator [edge -> dst slot]
                ind = work.tile([128, CMAX, 128], bf16, tag="ind")
                nc.vector.tensor_tensor(
                    ind[:, :CB, :],
                    iota_f[:, None, :].to_broadcast([128, CB, 128]),
                    dloc[:, :CB, None].to_broadcast([128, CB, 128]),
                    op=OP.is_equal,
                )

                # gather + per-sub-range scores/scale (SWDGE ring caps one
                # gather at 1024 idxs; each 8-chunk slice proceeds to
                # scores -> scale as soon as its gather lands)
                # sub-gather split: stay under the 1024-idx ring max
                # (896) and issue the small remainder first so the first
                # chunk's compute chain starts earliest
                GMAX = 6
                sizes = [((CB - 1) % GMAX) + 1]
                while sum(sizes) < CB:
                    sizes.append(GMAX)
                Gt = work.tile([128, CMAX, ROW], bf16, tag="G")
                Gf32 = Gt[:].bitcast(f32)
                exf = work.tile([128, CMAX, H], f32, tag="exf")
                exb = work.tile([128, CMAX, H], bf16, tag="exb")
                tmp = work.tile([128, CMAX, H], f32, tag="tmpe")
                c0 = 0
                for cw in sizes:
                    sl = slice(c0, c0 + cw)
                    nc.gpsimd.dma_gather(
                        Gt[:, sl, :], tbl[li][:, :],
                        sidx[:, c0 * 8 : (c0 + cw) * 8],
                        num_idxs=cw * 128, num_idxs_reg=cw * 128,
                        elem_size=ROW,
                    )
                    nc.vector.tensor_tensor(
                        exf[:, sl, :], Gf32[:, sl, C // 2 : C // 2 + H],
                        pdx[:, c0 * H : (c0 + cw) * H].rearrange(
                            "p (c h) -> p c h", h=H),
                        op=OP.add,
                    )
                    nc.scalar.mul(tmp[:, sl, :], exf[:, sl, :], NEG_SLOPE)
                    nc.vector.tensor_tensor(exf[:, sl, :], exf[:, sl, :],
                                            tmp[:, sl, :], op=OP.max)
                    nc.scalar.activation(exf[:, sl, :], exf[:, sl, :], AF.Exp)
                    nc.scalar.copy(exb[:, sl, :], exf[:, sl, :])
                    Gh = Gt[:, sl, :C].rearrange("p c (h f) -> p c h f",
                                                 f=C // H)
                    exv = exb[:, sl, :, None].to_broadcast(
                        [128, cw, H, C // H])
                    nc.vector.tensor_tensor(Gh, Gh, exv, op=OP.mult)
                    c0 += cw

                if debug and li == 0 and b == 0:
                    nc.sync.dma_start(dbg_G[:, :], Gt[:, :, :])
                    nc.sync.dma_start(dbg_ex[:, :], exf[:, :, :])
                    nc.sync.dma_start(dbg_ind[:, :], ind[:, :, :])

                # aggregate (den shares the agg tile's second PSUM bank)
                pagg = psum2.tile([128, C + H], f32, tag="agg")
                pa = pagg[:, :C]
                pd = pagg[:, C : C + H]
                for c in range(CB):
                    nc.tensor.matmul(pa, ind[:, c, :], Gt[:, c, :C],
                                     start=(c == 0), stop=(c == CB - 1))
                    nc.tensor.matmul(pd, ind[:, c, :], exb[:, c, :],
                                     start=(c == 0), stop=(c == CB - 1))

                # epilogue: fold in the self-loop term, then
                # h_new = relu(num/den + bias) (+ residual)
                nv = 128 if b < NBLK - 1 else LASTV
                exs = work.tile([128, H], f32, tag="exs")
                nc.vector.tensor_tensor(exs[:], s_sb[(li, b)][:],
                                        d_sb[(li, b)][:], op=OP.add)
                tmps = work.tile([128, H], f32, tag="tmps")
                nc.scalar.mul(tmps[:], exs[:], NEG_SLOPE)
                nc.vector.tensor_tensor(exs[:], exs[:], tmps[:], op=OP.max)
                nc.scalar.activation(exs[:], exs[:], AF.Exp)
                exs_b = work.tile([128, H], bf16, tag="exsb")
                nc.scalar.copy(exs_b[:], exs[:])
                hw_own = work.tile([128, C], bf16, tag="hwown")
                nc.vector.memset(hw_own[:], 0.0)
                nc.sync.dma_start(
                    hw_own[:nv, :],
                    tbl_sh[li][b * 128 : b * 128 + nv, :C],
                )
                selfm = work.tile([128, C], f32, tag="selfm")
                hwv = hw_own[:].rearrange("p (h f) -> p h f", f=C // H)
                exv2 = exs_b[:, :, None].to_broadcast([128, H, C // H])
                nc.vector.tensor_tensor(
                    selfm[:].rearrange("p (h f) -> p h f", f=C // H),
                    hwv, exv2, op=OP.mult,
                )
                nc.vector.tensor_tensor(selfm[:], selfm[:], pa, op=OP.add)
                rden = work.tile([128, H], f32, tag="rden")
                nc.vector.tensor_tensor(rden[:], pd, exs[:], op=OP.add)
                nc.vector.reciprocal(rden[:], rden[:])
                hn = hpool.tile([128, C], f32, tag=f"h{li}_{b}")
                HIDQ = C // H
                for h in range(H):
                    nc.scalar.mul(
                        hn[:, h * HIDQ : (h + 1) * HIDQ],
                        selfm[:, h * HIDQ : (h + 1) * HIDQ],
                        rden[:, h : h + 1],
                    )
                nc.vector.tensor_tensor(hn[:], hn[:], bias_t[li][:], op=OP.add)
                nc.scalar.activation(hn[:], hn[:], AF.Relu)
                if li > 0:
                    nc.vector.tensor_tensor(hn[:], hn[:],
                                            h_tiles[(li - 1, b)][:], op=OP.add)
                h_tiles[(li, b)] = hn
                if debug:
                    nc.sync.dma_start(dbg_h[li, b * 128 : (b + 1) * 128, :], hn[:])

                if li < 2:
                    # transpose to feature-major & next-layer projection
                    hb = work.tile([128, C], bf16, tag="hb16")
                    nc.scalar.copy(hb[:], hn[:])
                    hT = work.tile([128, KC, 128], bf16, tag="hT")
                    for f in range(KC):
                        pt = psum.tile([128, 128], bf16, tag="tr")
                        nc.tensor.transpose(
                            pt[:], hb[:, f * 128 : (f + 1) * 128], ident_b[:]
                        )
                        nc.scalar.copy(hT[:, f, :], pt[:])
                    hw_matmul_to_table(li + 1, b,
                                       [hT[:, f, :] for f in range(KC)])
                    maybe_allgather(li + 1, b)

        # ---------------- pooling + classifier ----------------
        # (rows beyond NPC in the last block hold finite garbage; their
        #  pool_oh one-hot rows are zero so they contribute nothing)
        pp = psum.tile([G, C], f32, tag="mm")
        for b in range(NBLK):
            nc.tensor.matmul(pp[:], poolOH_t[b][:], h_tiles[(2, b)][:],
                             start=(b == 0), stop=(b == NBLK - 1))
        sums = work.tile([G, C], f32, tag="sums")
        nc.vector.tensor_copy(sums[:], pp[:])
        nc.gpsimd.dma_start(pool_in[:, :], sums[:])
        nc.gpsimd.collective_compute(
            "AllReduce", OP.add, replica_groups=RG,
            ins=[pool_in[:, :]], outs=[pool_out[:, :]],
        )
        sfull = work.tile([G, C], f32, tag="sfull")
        nc.gpsimd.dma_start(sfull[:], pool_out[:, :])
        hp = work.tile([G, C], f32, tag="hp")
        nc.vector.tensor_scalar_mul(hp[:], sfull[:], invcnt_t[:, 0:1])

        hpT = work.tile([128, KC, G], f32, tag="hpT")
        for f in range(KC):
            pt = psum.tile([128, G], f32, tag="tr")
            nc.tensor.transpose(pt[:], hp[:, f * 128 : (f + 1) * 128],
                                ident_f[:G, :G])
            nc.vector.tensor_copy(hpT[:, f, :], pt[:])
        pc1 = psum.tile([G, 128], f32, tag="tr")
        for f in range(KC):
            nc.tensor.matmul(pc1[:], hpT[:, f, :], cW1_t[f][:],
                             start=(f == 0), stop=(f == KC - 1))
        z1 = work.tile([G, 128], f32, tag="z1")
        nc.vector.tensor_tensor(z1[:], pc1[:], cb1_t[:], op=OP.add)
        nc.scalar.activation(z1[:], z1[:], AF.Relu)
        pt2 = psum.tile([128, G], f32, tag="tr")
        nc.tensor.transpose(pt2[:], z1[:], ident_f[:G, :G])
        z1T = work.tile([128, G], f32, tag="z1T")
        nc.vector.tensor_copy(z1T[:], pt2[:])
        po = psum.tile([G, 1], f32, tag="dexp")
        nc.tensor.matmul(po[:], z1T[:], cW2_t[:], start=True, stop=True)
        outc = work.tile([G, 1], f32, tag="outc")
        nc.vector.tensor_scalar(outc[:], po[:], cb2_t[:, 0:1], None, op0=OP.add)

        nc.sync.dma_start(out_d[:, 0:1], outc[:])
        nc.sync.dma_start(out_d[:, 1 : C + 1], hp[:])

    return nc


# --------------------------------------------------------------------------
# Entry point
# --------------------------------------------------------------------------

def kernel(**inputs):
    x = np.asarray(inputs["x"], np.float32)
    edge_index = np.asarray(inputs["edge_index"])
    batch = np.asarray(inputs["batch"])
    cfg, in_maps = _prep(x, edge_index, batch, inputs)
    nc = _build(cfg)
    nc.compile()

    from concourse import bass_utils

    res = bass_utils.run_bass_kernel_spmd(
        nc, in_maps, core_ids=list(range(cfg["n_cores"]))
    ).results
    o = np.asarray(res[0]["out"], np.float32)
    out = np.ascontiguousarray(o[:, 0:1])
    hp = np.ascontiguousarray(o[:, 1 : cfg["C"] + 1])
    return out, hp
